# revision 4
# baseline (speedup 1.0000x reference)
"""Trainium2 Bass kernel for nn_BackboneCurvatureMixer.

Reference computation (per batch b, token t):
    z = h @ wred_w + wred_b                                   (B, L, 16)
    p_d[t] = normalize(antisym(z[t] (x) z[t+d]))[upper-tri]   d in {1,2,4}
    p_bb1 = p_1                                               (B, L, 120)
    kappa = p_1[t+1] - 2 p_1[t] + p_1[t-1]                    (B, L, 120)
    g_bb  = gelu(concat(p_1,p_2,p_4) @ bb_w1 + bb_b1) @ bb_w2 + bb_b2
    g_curv= gelu(kappa @ cv_w1 + cv_b1) @ cv_w2 + cv_b2
(seq_mask is all-ones per the problem spec; zero-padded z halos reproduce the
reference's edge behaviour exactly for the ones mask.)

Sharding: data-parallel over B across 8 cores (2 batches/core), SPMD, no
collectives.

On-chip layout: "Layout P" — features on SBUF partitions, tokens on the free
dim. h is transposed on the PE (128x128 fp32 transposes); z is computed
token-major (exact fp32) and transposed once into a batch-resident z_T row
tile; A/B = row-selections of z_T (fp32 PE matmuls); the antisymmetric
products run on DVE/GPSIMD; the norm uses gpsimd partition_all_reduce +
ACT exp(-0.5*ln(s+eps)); both MLPs run in fp32r (TF32-class) with
host-pre-rounded weights; layer 2 is computed token-major (lhsT = hidden
activations) so outputs DMA straight out without transposes.
"""
import os
import sys
import numpy as np

B, L, D, R = 16, 4096, 1024, 16
OFFSETS = (1, 2, 4)
PLU = R * (R - 1) // 2  # 120
HID = D // 2            # 512
NCORES = 8
BPC = B // NCORES       # batches per core
T = 512                 # tokens per tile
NT = L // T             # tiles per batch
NJ = L // 128           # 128-token tiles per batch (pass 1)
ZC = L + 8              # z_T columns: 1 left pad + L + 7 right pad
IU0, IU1 = np.triu_indices(R, 1)

_cache = {}


def _round_f32r(x):
    """fp32 -> fp32r rounding (round-to-nearest-even, 11 mantissa bits)."""
    b = np.ascontiguousarray(x, dtype=np.float32).view(np.uint32)
    shift = 23 - 11
    half = np.uint32(1 << (shift - 1))
    mask = np.uint32(~np.uint32((1 << shift) - 1))
    lsb = (b >> np.uint32(shift)) & np.uint32(1)
    out = ((b + half - np.uint32(1) + lsb) & mask).view(np.float32)
    return np.ascontiguousarray(out)


def _build():
    if '/opt/trn_rl_repo' not in sys.path:
        sys.path.insert(0, '/opt/trn_rl_repo')
    import concourse.bass as bass
    from concourse import bacc, mybir
    import concourse.tile as tile
    import concourse.bass_isa as bass_isa
    import concourse.tile_utils as tile_utils
    # 192KB is a stale cap (cayman has 208KB usable per partition)
    if getattr(tile_utils, 'max_sbuf_usage', 0) < 206 * 1024:
        tile_utils.max_sbuf_usage = 206 * 1024

    F32 = mybir.dt.float32
    F32R = mybir.dt.float32r
    AF = mybir.ActivationFunctionType
    RO = bass_isa.ReduceOp

    nc = bacc.Bacc(None, target_bir_lowering=False)

    # ---- DRAM parameters ----
    h_d = nc.declare_dram_parameter("h", [BPC, L, D], F32, isOutput=False)
    wredk_d = nc.declare_dram_parameter("wredk", [128, 128], F32, isOutput=False)
    wredb_d = nc.declare_dram_parameter("wredb", [1, 16], F32, isOutput=False)
    ones128_d = nc.declare_dram_parameter("ones128", [1, 128], F32, isOutput=False)
    ident_d = nc.declare_dram_parameter("ident", [128, 128], F32, isOutput=False)
    selA_d = nc.declare_dram_parameter("selA", [16, PLU], F32, isOutput=False)
    selB_d = nc.declare_dram_parameter("selB", [16, PLU], F32, isOutput=False)
    w1bb_d = nc.declare_dram_parameter("w1bb", [PLU, 3 * HID], F32R, isOutput=False)
    w1cv_d = nc.declare_dram_parameter("w1cv", [PLU, HID], F32R, isOutput=False)
    b1bb_d = nc.declare_dram_parameter("b1bb", [1, HID], F32R, isOutput=False)
    b1cv_d = nc.declare_dram_parameter("b1cv", [1, HID], F32R, isOutput=False)
    ones512_d = nc.declare_dram_parameter("ones512", [1, 512], F32R, isOutput=False)
    w2bb_d = nc.declare_dram_parameter("w2bb", [128, 4 * D], F32R, isOutput=False)
    w2cv_d = nc.declare_dram_parameter("w2cv", [128, 4 * D], F32R, isOutput=False)
    b2bb_d = nc.declare_dram_parameter("b2bb", [1, D], F32R, isOutput=False)
    b2cv_d = nc.declare_dram_parameter("b2cv", [1, D], F32R, isOutput=False)

    z_o = nc.declare_dram_parameter("z_o", [BPC, L, R], F32, isOutput=True)
    p_o = nc.declare_dram_parameter("p_o", [BPC, L, PLU], F32, isOutput=True)
    k_o = nc.declare_dram_parameter("k_o", [BPC, L, PLU], F32, isOutput=True)
    gb_o = nc.declare_dram_parameter("gb_o", [BPC, L, D], F32, isOutput=True)
    gc_o = nc.declare_dram_parameter("gc_o", [BPC, L, D], F32, isOutput=True)

    with tile.TileContext(nc) as tc:
        with (
            tc.tile_pool(name="consts", bufs=1) as cp,
            tc.tile_pool(name="zt", bufs=1) as ztp,
            tc.tile_pool(name="ab", bufs=1) as abp,
            tc.tile_pool(name="work", bufs=2) as wp,
            tc.tile_pool(name="tmp", bufs=1) as tp,
            tc.tile_pool(name="gp", bufs=8) as gp,
            tc.tile_pool(name="op", bufs=2) as op,
            tc.tile_pool(name="ps_ht", bufs=2, space="PSUM") as ps_ht,
            tc.tile_pool(name="ps_z", bufs=1, space="PSUM") as ps_z,
            tc.tile_pool(name="ps_ab", bufs=1, space="PSUM") as ps_ab,
            tc.tile_pool(name="ps_mm", bufs=3, space="PSUM") as ps_mm,
        ):
            # ---- constants into SBUF ----
            c_wred = cp.tile([128, 128], F32, tag="c_wred")
            nc.sync.dma_start(out=c_wred[:], in_=wredk_d.ap())
            c_wredb = cp.tile([1, 16], F32, tag="c_wredb")
            nc.sync.dma_start(out=c_wredb[:], in_=wredb_d.ap())
            c_ones128 = cp.tile([1, 128], F32, tag="c_ones128")
            nc.sync.dma_start(out=c_ones128[:], in_=ones128_d.ap())
            c_ident = cp.tile([128, 128], F32, tag="c_ident")
            nc.sync.dma_start(out=c_ident[:], in_=ident_d.ap())
            c_selA = cp.tile([16, PLU], F32, tag="c_selA")
            nc.sync.dma_start(out=c_selA[:], in_=selA_d.ap())
            c_selB = cp.tile([16, PLU], F32, tag="c_selB")
            nc.sync.dma_start(out=c_selB[:], in_=selB_d.ap())
            c_w1bb = cp.tile([PLU, 3 * HID], F32R, tag="c_w1bb")
            nc.sync.dma_start(out=c_w1bb[:], in_=w1bb_d.ap())
            c_w1cv = cp.tile([PLU, HID], F32R, tag="c_w1cv")
            nc.sync.dma_start(out=c_w1cv[:], in_=w1cv_d.ap())
            c_b1bb = cp.tile([1, HID], F32R, tag="c_b1bb")
            nc.sync.dma_start(out=c_b1bb[:], in_=b1bb_d.ap())
            c_b1cv = cp.tile([1, HID], F32R, tag="c_b1cv")
            nc.sync.dma_start(out=c_b1cv[:], in_=b1cv_d.ap())
            c_ones512 = cp.tile([1, 512], F32R, tag="c_ones512")
            nc.sync.dma_start(out=c_ones512[:], in_=ones512_d.ap())
            c_w2bb = cp.tile([128, 4 * D], F32R, tag="c_w2bb")
            nc.sync.dma_start(out=c_w2bb[:], in_=w2bb_d.ap())
            c_w2cv = cp.tile([128, 4 * D], F32R, tag="c_w2cv")
            nc.sync.dma_start(out=c_w2cv[:], in_=w2cv_d.ap())
            c_b2bb = cp.tile([1, D], F32R, tag="c_b2bb")
            nc.sync.dma_start(out=c_b2bb[:], in_=b2bb_d.ap())
            c_b2cv = cp.tile([1, D], F32R, tag="c_b2cv")
            nc.sync.dma_start(out=c_b2cv[:], in_=b2cv_d.ap())
            c_eps = cp.tile([128, 1], F32, tag="c_eps")
            nc.vector.memset(c_eps[:], 1e-16)

            for b in range(BPC):
                # ================= pass 1: z + z_T =================
                z_T = ztp.tile([16, ZC], F32, tag="z_T")
                nc.vector.memset(z_T[:, 0:1], 0.0)
                nc.vector.memset(z_T[:, 1 + L:ZC], 0.0)
                for j in range(NJ):
                    t0 = 128 * j
                    h_in = wp.tile([128, D], F32, tag="h_in")
                    nc.sync.dma_start(out=h_in[:], in_=h_d.ap()[b, t0:t0 + 128, :])
                    ht_sb = wp.tile([128, 8 * 128], F32, tag="ht_sb")
                    for g in range(2):
                        htp = ps_ht.tile([128, 512], F32, tag="htp")
                        for kk in range(4):
                            k = 4 * g + kk
                            nc.tensor.transpose(
                                htp[:, 128 * kk:128 * (kk + 1)],
                                h_in[:, 128 * k:128 * (k + 1)],
                                c_ident[:],
                            )
                        nc.scalar.copy(ht_sb[:, 512 * g:512 * (g + 1)], htp[:])
                    # z (token-major, exact fp32): out[tok, 16]
                    zq = ps_z.tile([128, 128], F32, tag="zq")
                    nc.tensor.matmul(zq[:, 0:16], lhsT=c_ones128[:],
                                     rhs=c_wredb[:], start=True, stop=False)
                    for k in range(8):
                        nc.tensor.matmul(
                            zq[:, 0:16],
                            lhsT=ht_sb[:, 128 * k:128 * (k + 1)],
                            rhs=c_wred[:, 16 * k:16 * (k + 1)],
                            start=False, stop=(k == 7),
                        )
                    z_tok = op.tile([128, 16], F32, tag="z_tok")
                    nc.scalar.copy(z_tok[:], zq[:, 0:16])
                    nc.scalar.dma_start(out=z_o.ap()[b, t0:t0 + 128, :],
                                        in_=z_tok[:])
                    zTps = ps_z.tile([16, 128], F32, tag="zTps")
                    nc.tensor.transpose(zTps[:], z_tok[:], c_ident[:])
                    nc.vector.tensor_copy(z_T[:, 1 + t0:1 + t0 + 128], zTps[:])

                # ================= pass 1b: A/B (row selections of z_T) ====
                A_sb = abp.tile([PLU, ZC], F32, tag="A_sb")
                B_sb = abp.tile([PLU, ZC], F32, tag="B_sb")
                for w0 in range(0, ZC, 512):
                    wN = min(512, ZC - w0)
                    for sel, dst in ((c_selA, A_sb), (c_selB, B_sb)):
                        sps = ps_ab.tile([PLU, 512], F32, tag="sps")
                        nc.tensor.matmul(sps[:, 0:wN], lhsT=sel[:],
                                         rhs=z_T[:, w0:w0 + wN],
                                         start=True, stop=True)
                        nc.scalar.copy(dst[:, w0:w0 + wN], sps[:, 0:wN])

                # ================= pass 2: per 512-token tile ==============
                for i in range(NT):
                    t0 = T * i
                    # --- plucker pre-products (cols: token t = t0-1+w for p1)
                    p_pre = {}
                    for d in OFFSETS:
                        if d == 1:
                            lo, n = t0, T + 2       # t in [t0-1, t0+513)
                        else:
                            lo, n = t0 + 1, T       # t in [t0, t0+512)
                        t1 = tp.tile([PLU, T + 2], F32, tag="t1")
                        t2 = tp.tile([PLU, T + 2], F32, tag="t2")
                        nc.vector.tensor_mul(t1[:, 0:n], A_sb[:, lo:lo + n],
                                             B_sb[:, lo + d:lo + d + n])
                        nc.gpsimd.tensor_mul(t2[:, 0:n], B_sb[:, lo:lo + n],
                                             A_sb[:, lo + d:lo + d + n])
                        pp = wp.tile([PLU, T + 2], F32, tag=f"pp{d}")
                        nc.vector.tensor_sub(pp[:, 0:n], t1[:, 0:n], t2[:, 0:n])
                        p_pre[d] = (pp, n)
                    # --- norms: f = exp(-0.5*ln(sum(p^2)+eps))
                    p_s = {}
                    for d in OFFSETS:
                        pp, n = p_pre[d]
                        sq = tp.tile([PLU, T + 2], F32, tag="sq")
                        nc.vector.tensor_mul(sq[:, 0:n], pp[:, 0:n], pp[:, 0:n])
                        sall = tp.tile([PLU, T + 2], F32, tag="sall")
                        nc.gpsimd.partition_all_reduce(
                            sall[:, 0:n], sq[:, 0:n], channels=PLU,
                            reduce_op=RO.add)
                        lnt = tp.tile([PLU, T + 2], F32, tag="lnt")
                        nc.scalar.activation(lnt[:, 0:n], sall[:, 0:n], AF.Ln,
                                             bias=c_eps[0:PLU])
                        fct = tp.tile([PLU, T + 2], F32, tag="fct")
                        nc.scalar.activation(fct[:, 0:n], lnt[:, 0:n], AF.Exp,
                                             scale=-0.5)
                        ps_t = wp.tile([PLU, T + 2], F32R, tag=f"ps{d}")
                        nc.vector.tensor_mul(ps_t[:, 0:n], pp[:, 0:n],
                                             fct[:, 0:n])
                        p_s[d] = ps_t
                    p1s = p_s[1]
                    p1f = p1s[:].bitcast(F32)
                    # --- kappa = p1[t+1] - 2 p1[t] + p1[t-1]
                    ksum = tp.tile([PLU, T], F32, tag="ksum")
                    nc.vector.tensor_add(ksum[:], p1f[:, 0:T], p1f[:, 2:T + 2])
                    km2 = tp.tile([PLU, T], F32, tag="km2")
                    nc.vector.tensor_scalar_mul(km2[:], p1f[:, 1:T + 1], -2.0)
                    kap = wp.tile([PLU, T], F32R, tag="kap")
                    nc.vector.tensor_add(kap[:], ksum[:], km2[:])

                    # --- MLPs (fp32r) ---
                    for mlp, w1, b1, w2, b2, nk, go in (
                        ("bb", c_w1bb, c_b1bb, c_w2bb, c_b2bb, 3, gb_o),
                        ("cv", c_w1cv, c_b1cv, c_w2cv, c_b2cv, 1, gc_o),
                    ):
                        g_sb = []
                        for m in range(4):
                            gps = ps_mm.tile([128, 512], F32, tag="mm")
                            nc.tensor.matmul(
                                gps[:], lhsT=b1[:, 128 * m:128 * (m + 1)],
                                rhs=c_ones512[:], start=True, stop=False)
                            if mlp == "bb":
                                for di, d in enumerate(OFFSETS):
                                    nc.tensor.matmul(
                                        gps[:],
                                        lhsT=w1[:, 512 * di + 128 * m:
                                                512 * di + 128 * (m + 1)],
                                        rhs=p_s[d][:, 1:T + 1] if d == 1
                                        else p_s[d][:, 0:T],
                                        start=False, stop=(di == nk - 1))
                            else:
                                nc.tensor.matmul(
                                    gps[:], lhsT=w1[:, 128 * m:128 * (m + 1)],
                                    rhs=kap[:], start=False, stop=True)
                            gt = gp.tile([128, 512], F32R, tag="g")
                            nc.scalar.activation(gt[:], gps[:], AF.Gelu)
                            g_sb.append(gt)
                        for u in range(4):
                            out_sb = op.tile([128, D], F32, tag="out_sb")
                            for nck in range(2):
                                ops_ = ps_mm.tile([128, 512], F32, tag="mm")
                                nc.tensor.matmul(
                                    ops_[:], lhsT=c_ones512[:, 0:128],
                                    rhs=b2[:, 512 * nck:512 * (nck + 1)],
                                    start=True, stop=False)
                                for m in range(4):
                                    nc.tensor.matmul(
                                        ops_[:],
                                        lhsT=g_sb[m][:, 128 * u:128 * (u + 1)],
                                        rhs=w2[:, D * m + 512 * nck:
                                               D * m + 512 * (nck + 1)],
                                        start=False, stop=(m == 3))
                                if nck == 0:
                                    nc.vector.tensor_copy(
                                        out_sb[:, 0:512], ops_[:])
                                else:
                                    nc.scalar.copy(out_sb[:, 512:1024], ops_[:])
                            nc.sync.dma_start(
                                out=go.ap()[b, t0 + 128 * u:t0 + 128 * (u + 1), :],
                                in_=out_sb[:])

                    # --- p_bb1 / kappa outputs (token-major via PE transpose)
                    for src, dst_d, is_r in ((p1s, p_o, True), (kap, k_o, True)):
                        tps = ps_ht.tile([128, 512], F32, tag="htp")
                        for u in range(4):
                            sl = (src[:, 1 + 128 * u:1 + 128 * (u + 1)]
                                  if src is p1s else
                                  src[:, 128 * u:128 * (u + 1)])
                            nc.tensor.transpose(
                                tps[:, PLU * u:PLU * (u + 1)],
                                sl.bitcast(F32), c_ident[0:PLU, 0:PLU])
                        pk_sb = op.tile([128, 4 * PLU], F32, tag="pk_sb")
                        nc.scalar.copy(pk_sb[:], tps[:, 0:4 * PLU])
                        nc.scalar.dma_start(
                            out=dst_d.ap()[b, t0:t0 + T, :].rearrange(
                                "(u p) k -> p u k", p=128),
                            in_=pk_sb[:].rearrange("p (u k) -> p u k", u=4))

    nc.finalize()
    return nc


def _prep_consts(inputs):
    wred_w = inputs["wred_w"]
    c = {}
    c["wredk"] = np.ascontiguousarray(
        wred_w.reshape(8, 128, 16).transpose(1, 0, 2).reshape(128, 128))
    c["wredb"] = np.ascontiguousarray(inputs["wred_b"].reshape(1, 16))
    c["ones128"] = np.ones((1, 128), np.float32)
    c["ident"] = np.eye(128, dtype=np.float32)
    selA = np.zeros((16, PLU), np.float32)
    selB = np.zeros((16, PLU), np.float32)
    selA[IU0, np.arange(PLU)] = 1.0
    selB[IU1, np.arange(PLU)] = 1.0
    c["selA"], c["selB"] = selA, selB
    c["w1bb"] = _round_f32r(
        inputs["bb_w1"].reshape(3, PLU, HID).transpose(1, 0, 2).reshape(PLU, 3 * HID))
    c["w1cv"] = _round_f32r(inputs["cv_w1"])
    c["b1bb"] = _round_f32r(inputs["bb_b1"].reshape(1, HID))
    c["b1cv"] = _round_f32r(inputs["cv_b1"].reshape(1, HID))
    c["ones512"] = np.ones((1, 512), np.float32)
    c["w2bb"] = _round_f32r(
        inputs["bb_w2"].reshape(4, 128, D).transpose(1, 0, 2).reshape(128, 4 * D))
    c["w2cv"] = _round_f32r(
        inputs["cv_w2"].reshape(4, 128, D).transpose(1, 0, 2).reshape(128, 4 * D))
    c["b2bb"] = _round_f32r(inputs["bb_b2"].reshape(1, D))
    c["b2cv"] = _round_f32r(inputs["cv_b2"].reshape(1, D))
    return c


last_exec_time_ns = None


def kernel(**inputs):
    global last_exec_time_ns
    if '/opt/trn_rl_repo' not in sys.path:
        sys.path.insert(0, '/opt/trn_rl_repo')
    from concourse.bass_utils import run_bass_kernel_spmd

    if 'nc' not in _cache:
        _cache['nc'] = _build()
    nc = _cache['nc']

    consts = _prep_consts(inputs)
    h = np.ascontiguousarray(inputs["h"], dtype=np.float32)
    in_maps = []
    for core in range(NCORES):
        m = dict(consts)
        m["h"] = np.ascontiguousarray(h[core * BPC:(core + 1) * BPC])
        in_maps.append(m)

    trace = bool(int(os.environ.get("BASS_KERNEL_TRACE", "0")))
    res = run_bass_kernel_spmd(nc, in_maps, list(range(NCORES)), trace=trace)
    last_exec_time_ns = res.exec_time_ns

    z = np.concatenate([r["z_o"] for r in res.results], axis=0)
    p = np.concatenate([r["p_o"] for r in res.results], axis=0)
    k = np.concatenate([r["k_o"] for r in res.results], axis=0)
    gb = np.concatenate([r["gb_o"] for r in res.results], axis=0)
    gc = np.concatenate([r["gc_o"] for r in res.results], axis=0)
    return (z, gb, gc, p, k)
